# revision 40
# baseline (speedup 1.0000x reference)
"""Distributed GQA attention (RoPE + causal mask + o_proj) on 8 TRN2 NeuronCores.

Sharding: 8-way tensor parallel over heads. Core c handles q heads
[4c, 4c+4) and kv head c (the matching GQA group) for BOTH batches.

v2 structure (vs the v1 baseline):
  - projections contract dc-outer so the PE starts ~3us in and each
    weight chunk's LDWEIGHTS is reused across 4-6 matmuls
  - score matmuls for the two heads of a pair are emitted interleaved so
    their (0,0)/(64,0) PE row-tiles execute concurrently (2x score rate)
  - per-head-pair exp runs as two [128,1024] ACTIVATEs per k-tile-pair;
    the next pair's score matmuls are emitted BEFORE this pair's AV
    matmuls so ScalarE (the attention pace-setter) never starves
  - softmax normalization reads the PSUM accumulator directly
    (reciprocal + gpsimd partition broadcast + one DVE multiply)
  - o_proj for batch 0 is emitted interleaved into batch 1's attention
    loops, filling the PE bubbles left by the exp dependency chain
  - ScalarE runs ONLY exp; all copies live on VectorE; DMA issue is
    spread across the sync/gpsimd/vector queues
"""

import os
import sys
import math

for _p in ("/opt/trn_rl_repo", "/root/.axon_site/_ro/trn_rl_repo"):
    if os.path.isdir(_p) and _p not in sys.path:
        sys.path.append(_p)

import numpy as np
import ml_dtypes

import concourse.bass as bass
import concourse.bacc as bacc
import concourse.tile as tile
import concourse.mybir as mybir
from concourse import bass_utils

FP32 = mybir.dt.float32
BF16 = mybir.dt.bfloat16
AF = mybir.ActivationFunctionType
ALU = mybir.AluOpType
PSUM = bass.MemorySpace.PSUM
NPBF16 = ml_dtypes.bfloat16

B = 2
D = 2048
S = 2048
HD = 64
N_HEADS = 32
N_KV = 8
NCORES = 8
HQ = N_HEADS // NCORES   # 4 local q heads
QCOLS = HQ * HD          # 256
NDC = D // 128           # 16 contraction chunks
NPAIR = HQ // 2          # 2 head pairs
THETA = 10000.0


def build_graph(s: int = S):
    """Build + compile the per-core SPMD graph (causal only)."""
    bs = B * s               # flattened rows
    nqb = s // 512           # q blocks per batch
    nkt = s // 128           # k tiles per batch
    rows_h = s // NCORES     # output rows per core per batch
    rt_m = min(128, rows_h)  # o_proj row-tile height
    nrt_h = rows_h // rt_m   # o_proj row tiles per batch half

    nc = bacc.Bacc("TRN2", target_bir_lowering=False, debug=False,
                   enable_asserts=True, num_devices=NCORES)

    xT_h = nc.dram_tensor("xT", [D, bs], BF16, kind="ExternalInput")
    wq_h = nc.dram_tensor("wq", [D, QCOLS], BF16, kind="ExternalInput")
    wkv_h = nc.dram_tensor("wkv", [D, 2 * HD], BF16, kind="ExternalInput")
    wo_h = nc.dram_tensor("wo", [D, D], BF16, kind="ExternalInput")
    c4_h = nc.dram_tensor("c4", [128, s], FP32, kind="ExternalInput")
    s4_h = nc.dram_tensor("s4", [128, s], FP32, kind="ExternalInput")
    id_h = nc.dram_tensor("ident", [128, 128], BF16, kind="ExternalInput")
    pw_h = nc.dram_tensor("pswap", [128, 128], BF16, kind="ExternalInput")
    md_h = nc.dram_tensor("mdiag", [128, 1024], BF16, kind="ExternalInput")
    out_h = nc.dram_tensor("out", [B * rows_h, D], FP32, kind="ExternalOutput")

    with tile.TileContext(nc) as tc:
        with tc.tile_pool(name="persist", bufs=1) as pp, \
             tc.tile_pool(name="dram", bufs=1, space="DRAM") as dramp:

            # ---- constants (tiles only; DMAs issued after the first x/w
            # chunk loads so they don't steal startup HBM bandwidth) ----
            c4 = pp.tile([128, s], FP32, tag="c4", name="c4t")
            s4 = pp.tile([128, s], FP32, tag="s4", name="s4t")
            ident = pp.tile([128, 128], BF16, tag="idb", name="identb")
            psw = pp.tile([128, 128], BF16, tag="pwb", name="pswapb")
            md = pp.tile([128, 1024], BF16, tag="mdb", name="mdb")

            def load_constants():
                # scalar DMA queue: idle during projections
                nc.scalar.dma_start(ident[:, :], id_h[:, :])
                nc.scalar.dma_start(psw[:, :], pw_h[:, :])
                nc.scalar.dma_start(c4[:, :], c4_h[:, :])
                nc.scalar.dma_start(s4[:, :], s4_h[:, :])
                nc.scalar.dma_start(md[:, :], md_h[:, :])

            # warm the exp table set early so the first attention exp
            # doesn't eat the ~2.7us ACT_TABLE_LOAD
            warm = pp.tile([1, 64], FP32, tag="warm", name="warm")
            nc.vector.memset(warm[0:1, :], 0.0)
            nc.scalar.activation(warm[0:1, :], warm[0:1, :], AF.Exp)
            # [1, 64] ones: stationary for the K=1 broadcast matmuls that
            # replicate the softmax reciprocals across partitions
            ones = pp.tile([1, HD], BF16, tag="ones", name="ones")
            nc.vector.memset(ones[0:1, :], 1.0)

            # ---- persistent activations ----
            qT = [pp.tile([128, bs], BF16, tag=f"qT{i}", name=f"qT{i}")
                  for i in range(NPAIR)]
            kTrep = [pp.tile([128, s], BF16, tag=f"kTr{i}", name=f"kTrep{i}")
                     for i in range(B)]
            vb = [[pp.tile([128, HD + 1], BF16, tag=f"vb{b}_{i}",
                           name=f"vb{b}_{i}") for i in range(nkt)]
                  for b in range(B)]
            # attn^T, one [64, bs] tile per local head (partition base 0)
            attnT = [pp.tile([64, bs], BF16, tag=f"aT{i}", name=f"attnT{i}")
                     for i in range(HQ)]

            bnc_in = [[dramp.tile([NCORES, 2 * HD, rows_h], BF16,
                                  tag=f"bin{b}_{hp}", name=f"bounce_in{b}_{hp}")
                       for hp in range(NPAIR)] for b in range(B)]
            bnc_out = [[dramp.tile([NCORES, 2 * HD, rows_h], BF16,
                                   tag=f"bout{b}_{hp}", name=f"bounce_out{b}_{hp}")
                        for hp in range(NPAIR)] for b in range(B)]

            # ================= phase P: projections + RoPE =================
            def rope(src_ps, swap_ps, scale, dst_ap, sl0, npart, rp, nm):
                """dst = (src * scale) .* c4 + (swap(src) * scale) .* s4."""
                cs = c4[0:npart, sl0:sl0 + 512]
                sn = s4[0:npart, sl0:sl0 + 512]
                m1 = rp.tile([npart, 512], FP32, tag=f"m1_{npart}", name=f"m1{nm}")
                nc.vector.scalar_tensor_tensor(m1[:, :], src_ps, scale, cs,
                                               ALU.mult, ALU.mult)
                m2 = rp.tile([npart, 512], FP32, tag=f"m2_{npart}", name=f"m2{nm}")
                nc.vector.scalar_tensor_tensor(m2[:, :], swap_ps, scale, sn,
                                               ALU.mult, ALU.mult)
                nc.vector.tensor_add(dst_ap, m1[:, :], m2[:, :])

            with tc.tile_pool(name="wres", bufs=1) as wrp, \
                 tc.tile_pool(name="xpool", bufs=18) as xp, \
                 tc.tile_pool(name="ropes", bufs=3) as rp, \
                 tc.tile_pool(name="qps_p", bufs=1, space=PSUM) as pq, \
                 tc.tile_pool(name="qsw_p", bufs=1, space=PSUM) as pqs, \
                 tc.tile_pool(name="kv_p", bufs=1, space=PSUM) as pkv, \
                 tc.tile_pool(name="tp_p", bufs=1, space=PSUM) as ptp:

                # per-dc weight chunk loads: small DMAs so the first matmul
                # can start ~3us in; spread across two queues
                wq_all = wrp.tile([128, NDC * QCOLS], BF16, tag="wqa",
                                  name="wq_all")
                wkv_all = wrp.tile([128, NDC * 2 * HD], BF16, tag="wkva",
                                   name="wkv_all")
                wqb = [wq_all[:, dc * QCOLS:(dc + 1) * QCOLS]
                       for dc in range(NDC)]
                wkvb = [wkv_all[:, dc * 2 * HD:(dc + 1) * 2 * HD]
                        for dc in range(NDC)]

                nxp = bs // 1024      # 1024-row compute slabs (2 rs slices)
                xtb2 = {}             # xj -> 16 [128, 2048] tiles (2 slabs)
                for xi in range(nxp):
                    x0 = xi * 1024
                    if xi % 2 == 0:
                        xj = xi // 2
                        tl = []
                        for dc in range(NDC):
                            if xi == 0:
                                # interleave weight chunks with x tiles so
                                # the dc-outer matmul stream starts at once
                                nc.sync.dma_start(
                                    wkvb[dc], wkv_h[dc * 128:(dc + 1) * 128, :])
                                nc.gpsimd.dma_start(
                                    wqb[dc], wq_h[dc * 128:(dc + 1) * 128, :])
                            xt = xp.tile([128, 2048], BF16, tag="xtb",
                                         name=f"xtb2_{xj}_{dc}")
                            eng = nc.sync if dc % 2 == 0 else nc.gpsimd
                            eng.dma_start(
                                xt[:, :],
                                xT_h[dc * 128:(dc + 1) * 128,
                                     xj * 2048:(xj + 1) * 2048])
                            tl.append(xt)
                        if xi == 0:
                            load_constants()
                        xtb2[xj] = tl
                    xh = (xi % 2) * 1024
                    xtb = [t[:, xh:xh + 1024] for t in xtb2[xi // 2]]

                    # dc-outer accumulation: all psum tiles live across the
                    # dc loop; each stationary is loaded once per dc
                    kvps = [pkv.tile([128, 512], FP32, tag=f"kvps{sub}",
                                     name=f"kvps{xi}_{sub}") for sub in range(2)]
                    qps = [[pq.tile([128, 512], FP32, tag=f"qps{hp}_{sub}",
                                    name=f"qps{xi}_{hp}_{sub}")
                            for sub in range(2)] for hp in range(NPAIR)]
                    for dc in range(NDC):
                        st = (dc == 0)
                        sp = (dc == NDC - 1)
                        for sub in range(2):
                            xsl = slice(sub * 512, sub * 512 + 512)
                            nc.tensor.matmul(kvps[sub][:, :], wkvb[dc],
                                             xtb[dc][:, xsl], start=st, stop=sp)
                        for hp in range(NPAIR):
                            for sub in range(2):
                                xsl = slice(sub * 512, sub * 512 + 512)
                                nc.tensor.matmul(
                                    qps[hp][sub][:, :],
                                    wqb[dc][:, hp * 128:(hp + 1) * 128],
                                    xtb[dc][:, xsl], start=st, stop=sp)

                    for sub in range(2):
                        rs = xi * 2 + sub
                        r0 = rs * 512          # flattened row offset
                        b = r0 // s            # batch of this slice
                        sl0 = r0 - b * s       # seq offset within batch

                        # k: rope + replicate into both 64-partition halves
                        ksb = rp.tile([64, 512], BF16, tag="ksb", name=f"ksb{rs}")
                        nc.vector.tensor_copy(ksb[:, :], kvps[sub][0:64, :])
                        ksw = pqs.tile([64, 512], FP32, tag="qsw", name=f"ksw{rs}")
                        nc.tensor.matmul(ksw[:, :], psw[0:64, 0:64], ksb[:, :],
                                         start=True, stop=True)
                        ktmp = rp.tile([64, 512], BF16, tag="ktmp", name=f"ktmp{rs}")
                        rope(kvps[sub][0:64, :], ksw[:, :], 1.0, ktmp[:, :],
                             sl0, 64, rp, f"k{rs}")
                        nc.sync.dma_start(kTrep[b][0:64, sl0:sl0 + 512], ktmp[:, :])
                        nc.sync.dma_start(kTrep[b][64:128, sl0:sl0 + 512], ktmp[:, :])
                        # v: PE-transpose vT to row-major + ones col
                        vts = rp.tile([128, 512], BF16, tag="vts", name=f"vts{rs}")
                        nc.vector.tensor_copy(vts[64:128, :], kvps[sub][64:128, :])
                        for rb in range(4):
                            kt = sl0 // 128 + rb
                            tps = ptp.tile([128, HD], BF16, tag="tps",
                                           name=f"vt{rs}_{rb}")
                            nc.tensor.transpose(tps[:, :],
                                                vts[64:128, rb * 128:(rb + 1) * 128],
                                                ident[64:128, 64:128])
                            nc.vector.tensor_copy(vb[b][kt][:, 0:HD], tps[:, :])
                            nc.vector.memset(vb[b][kt][:, HD:HD + 1], 1.0)

                        # q projection rope, one head pair at a time
                        for hp in range(NPAIR):
                            qsb = rp.tile([128, 512], BF16, tag="qsb",
                                          name=f"qsb{rs}_{hp}")
                            nc.vector.tensor_copy(qsb[:, :], qps[hp][sub][:, :])
                            qsw = pqs.tile([128, 512], FP32, tag="qsw",
                                           name=f"qsw{rs}_{hp}")
                            nc.tensor.matmul(qsw[:, :], psw[:, :], qsb[:, :],
                                             start=True, stop=True)
                            rope(qps[hp][sub][:, :], qsw[:, :], 0.125,
                                 qT[hp][:, r0:r0 + 512], sl0, 128, rp,
                                 f"q{rs}_{hp}")

            # wo tiles prefetch during attention (no deps besides SBUF slots)
            wop_ctx = tc.tile_pool(name="wob", bufs=1)
            wop = wop_ctx.__enter__()
            wot = []
            for ch in range(NDC):
                wt = wop.tile([128, D], BF16, tag=f"wob{ch}", name=f"wob{ch}")
                nc.sync.dma_start(wt[:, :], wo_h[ch * 128:(ch + 1) * 128, :])
                wot.append(wt)

            # ============ attention (+ per-batch-pair A2A) + o_proj ============
            with tc.tile_pool(name="sc_p", bufs=1, space=PSUM) as psc, \
                 tc.tile_pool(name="at_p", bufs=1, space=PSUM) as pat, \
                 tc.tile_pool(name="y_p", bufs=1, space=PSUM) as pyo, \
                 tc.tile_pool(name="rc_p", bufs=1, space=PSUM) as prc, \
                 tc.tile_pool(name="probs", bufs=3) as prp, \
                 tc.tile_pool(name="att2", bufs=1) as a2p, \
                 tc.tile_pool(name="yout", bufs=2) as yop, \
                 tc.tile_pool(name="norm", bufs=2) as nrm:

                def sc_mms(b, hp, qb, kp):
                    """4 score MMs for one k-pair. par-inner adjacency with
                    explicit tile_position so the (0,0)/(64,0) PE row-tile
                    pairs execute concurrently. Returns the two tiles."""
                    q0 = qb * 512
                    g0 = b * s + q0
                    out = [psc.tile([128, 1024], FP32, tag=f"sc{par}",
                                    name=f"sc{par}_{b}_{qb}_{hp}_{kp}")
                           for par in range(2)]
                    for i in range(2):
                        kt = 2 * kp + i
                        k0 = kt * 128
                        for par in range(2):
                            pr = par * 64
                            nc.tensor.matmul(
                                out[par][:, i * 512:(i + 1) * 512],
                                kTrep[b][pr:pr + 64, k0:k0 + 128],
                                qT[hp][pr:pr + 64, g0:g0 + 512],
                                start=True, stop=True,
                                tile_position=(pr, 0))
                    return out

                def attn_block(b, hp, qb, sc0, next_sc, filler=None):
                    """Emit one q-block of attention for head pair hp.

                    sc0: pre-emitted score tiles for this block's kp=0 (the
                    predecessor emitted them so ScalarE never bubbles at
                    block boundaries). next_sc() emits the successor
                    block's kp=0 right after this block's last exp.
                    filler() slots a few independent PE instructions
                    (o_proj) into the exp dependency gaps each iteration.
                    """
                    q0 = qb * 512            # seq offset within batch
                    g0 = b * s + q0          # flattened offset
                    kt_end = 4 * (qb + 1)
                    nkp = kt_end // 2
                    acc = [pat.tile([HD + 1, 512], FP32, tag=f"a{par}",
                                    name=f"a{par}_{b}_{qb}_{hp}")
                           for par in range(2)]
                    sc = sc0
                    for kp in range(nkp):
                        pbt = prp.tile([128, 2048], BF16, tag="pb",
                                       name=f"pb{b}_{qb}_{hp}_{kp}")
                        for par in range(2):
                            nc.scalar.activation(
                                pbt[:, par * 1024:(par + 1) * 1024],
                                sc[par][:, :], AF.Exp)
                        # prefetch the next k-pair's scores (or the next
                        # block's first) now that this pair's psum is free
                        if kp + 1 < nkp:
                            sc = sc_mms(b, hp, qb, kp + 1)
                        elif next_sc is not None:
                            next_sc()
                        for i in range(2):
                            kt = 2 * kp + i
                            if kt >= 4 * qb:      # diagonal band: 0/1 mask
                                delta = kt * 128 - q0
                                msl = md[:, 512 - delta:1024 - delta]
                                for par in range(2):
                                    c0 = par * 1024 + i * 512
                                    nc.vector.tensor_mul(
                                        pbt[:, c0:c0 + 512],
                                        pbt[:, c0:c0 + 512], msl)
                        for par in range(2):
                            for i in range(2):
                                kt = 2 * kp + i
                                c0 = par * 1024 + i * 512
                                nc.tensor.matmul(
                                    acc[par][:, :], vb[b][kt][:, :],
                                    pbt[:, c0:c0 + 512],
                                    start=(kt == 0), stop=(kt == kt_end - 1))
                        if filler is not None:
                            filler()

                    # normalization: DVE only copies PSUM out (releasing the
                    # acc banks fast) + tiny reciprocal; the broadcast and
                    # the big multiply run on GpSimd so the DVE queue never
                    # blocks on a cross-engine dependency
                    # normalization: DVE copies PSUM out (releasing the acc
                    # banks fast) + reciprocal; a K=1 PE matmul broadcasts
                    # the reciprocal across 64 partitions into PSUM (keeps
                    # gpsimd - busy with collectives - off this path)
                    asb, rdb = [], []
                    for par in range(2):
                        a = nrm.tile([HD, 512], FP32, tag=f"asb{par}",
                                     name=f"asb{b}_{qb}_{hp}_{par}")
                        nc.vector.tensor_copy(a[:, :], acc[par][0:HD, :])
                        rs = nrm.tile([1, 512], FP32, tag=f"rds{par}",
                                      name=f"rds{b}_{qb}_{hp}_{par}")
                        nc.vector.tensor_copy(rs[0:1, :], acc[par][HD:HD + 1, :])
                        r = nrm.tile([1, 512], BF16, tag=f"rd{par}",
                                     name=f"rd{b}_{qb}_{hp}_{par}")
                        rf = nrm.tile([1, 512], FP32, tag=f"rdf{par}",
                                      name=f"rdf{b}_{qb}_{hp}_{par}")
                        nc.vector.reciprocal_approx_fast(rf[0:1, :], rs[0:1, :])
                        nc.vector.tensor_copy(r[0:1, :], rf[0:1, :])
                        asb.append(a)
                        rdb.append(r)
                    rcb = prc.tile([128, 512], FP32, tag="rcb",
                                   name=f"rcb{b}_{qb}_{hp}")
                    for par in range(2):
                        nc.tensor.matmul(rcb[par * 64:(par + 1) * 64, :],
                                         ones[0:1, :], rdb[par][0:1, :],
                                         start=True, stop=True,
                                         tile_position=(0, par * 64))
                    for par in range(2):
                        head = hp * 2 + par
                        nc.vector.tensor_mul(attnT[head][:, g0:g0 + 512],
                                             asb[par][0:HD, :],
                                             rcb[par * 64:(par + 1) * 64, :])
                    # stage this q-block's two core-row slices for the A2A
                    for j in (2 * qb, 2 * qb + 1):
                        for par in range(2):
                            head = hp * 2 + par
                            nc.sync.dma_start(
                                bnc_in[b][hp][j, par * 64:(par + 1) * 64, :],
                                attnT[head][:, b * s + j * rows_h:
                                            b * s + (j + 1) * rows_h])

                def a2a(b, hp):
                    nc.gpsimd.collective_compute(
                        "AllToAll", ALU.bypass,
                        replica_groups=[list(range(NCORES))],
                        ins=[bnc_in[b][hp].opt()],
                        outs=[bnc_out[b][hp].opt()],
                    )

                att2 = {}

                def att2_load(b):
                    tiles = []
                    for i in range(NCORES):
                        for hp in range(NPAIR):
                            t = a2p.tile([128, rows_h], BF16,
                                         tag=f"at2_{b}_{i}_{hp}",
                                         name=f"att2_{b}_{i}_{hp}")
                            # gpsimd queue: the wait on the collective must
                            # not sit in front of DMAs other engines' deps
                            # are semaphore-coupled to (sync queue)
                            nc.gpsimd.dma_start(t[:, :], bnc_out[b][hp][i, :, :])
                            tiles.append(t)
                    att2[b] = tiles

                ngrp = (D // 512) * nrt_h      # o_proj groups per batch

                def oproj_steps(b):
                    """Generator: o_proj matmul micro-steps for batch b.
                    Each yield is one matmul; group boundaries allocate the
                    PSUM tile and emit the output store."""
                    for g in range(ngrp):
                        oc, rt = g // nrt_h, g % nrt_h
                        o0 = oc * 512
                        yps = pyo.tile([rt_m, 512], FP32, tag="y",
                                       name=f"y{b}_{oc}_{rt}")
                        for ch in range(NDC):
                            nc.tensor.matmul(
                                yps[:, :],
                                att2[b][ch][:, rt * rt_m:(rt + 1) * rt_m],
                                wot[ch][:, o0:o0 + 512],
                                start=(ch == 0), stop=(ch == NDC - 1))
                            yield
                        ysb = yop.tile([rt_m, 512], FP32, tag="ysb",
                                       name=f"ysb{b}_{oc}_{rt}")
                        nc.vector.tensor_copy(ysb[:, :], yps[:, :])
                        nc.sync.dma_start(
                            out_h[b * rows_h + rt * rt_m:
                                  b * rows_h + (rt + 1) * rt_m, o0:o0 + 512],
                            ysb[:, :])

                # batch-0 o_proj matmuls slot into batch-1 attention's PE
                # gaps (4 per k-pair step, after a delay so the A2A and
                # att2 loads have safely landed)
                osteps = oproj_steps(0)
                kp_slot = [0]

                def filler():
                    # disabled: interleaving o_proj into attention couples
                    # the PE queue to the collectives' completion and the
                    # induced cross-core skew costs more than the overlap
                    # saves; o_proj b0 instead runs contiguously after the
                    # last attention block, overlapping A2A(b1,hp1)
                    kp_slot[0] += 1

                blocks = [(b, hp, qb) for b in range(B)
                          for hp in range(NPAIR) for qb in range(nqb)]
                sc_pending = {0: sc_mms(*blocks[0], 0)}

                def make_next_sc(idx):
                    if idx + 1 >= len(blocks):
                        return None

                    def f():
                        nb, nhp, nq = blocks[idx + 1]
                        sc_pending[idx + 1] = sc_mms(nb, nhp, nq, 0)
                    return f

                for idx, (b, hp, qb) in enumerate(blocks):
                    attn_block(b, hp, qb, sc_pending.pop(idx),
                               make_next_sc(idx),
                               filler if b == 1 else None)
                    if qb == nqb - 1:
                        a2a(b, hp)
                        if b == 0 and hp == NPAIR - 1:
                            att2_load(0)
                for _ in osteps:               # drain any remainder
                    pass

                att2_load(1)
                for _ in oproj_steps(1):
                    pass

            wop_ctx.__exit__(None, None, None)

    nc.compile()
    return nc


# ===================== host side =====================

def _rope_tables(s):
    freqs = THETA ** (-np.arange(0, HD, 2, dtype=np.float64) / HD)   # [32]
    ang = np.arange(s, dtype=np.float64)[:, None] * freqs[None, :]   # [s, 32]
    cosT = np.cos(ang).T.astype(np.float32)                          # [32, s]
    sinT = np.sin(ang).T.astype(np.float32)
    c4 = np.tile(cosT, (4, 1))                                       # [128, s]
    s4 = np.tile(np.concatenate([-sinT, sinT], axis=0), (2, 1))      # [128, s]
    return np.ascontiguousarray(c4), np.ascontiguousarray(s4)


def _pswap():
    # permutation matrix: swap 32-halves within each 64 block (symmetric)
    p = np.zeros((128, 128), dtype=np.float32)
    for blk in range(2):
        for i in range(32):
            p[blk * 64 + i, blk * 64 + 32 + i] = 1.0
            p[blk * 64 + 32 + i, blk * 64 + i] = 1.0
    return p


def _mdiag():
    # keep[p, u] = 1 iff u >= p + 512 (sliced per diagonal tile offset)
    u = np.arange(1024)[None, :]
    p = np.arange(128)[:, None]
    return (u >= p + 512).astype(np.float32)


def _perm_even_odd(w, n_heads_w):
    # reorder each head's 64 columns: even indices first, then odd
    perm = np.concatenate([np.arange(0, HD, 2), np.arange(1, HD, 2)])
    wr = w.reshape(D, n_heads_w, HD)[:, :, perm]
    return np.ascontiguousarray(wr.reshape(D, n_heads_w * HD))


def _is_causal(mask, s):
    m = np.asarray(mask, dtype=np.float32).reshape(s, s)
    tri = np.tril(np.ones((s, s), dtype=bool))
    return bool(np.all(m[tri] == 0.0) and np.all(m[~tri] <= -1e8))


def _bf16(a):
    return np.ascontiguousarray(np.asarray(a, np.float32).astype(NPBF16))


def make_in_maps(x, mask, wq, wk, wv, wo, s=S):
    """Shard full inputs into 8 per-core input dicts."""
    c4, s4 = _rope_tables(s)
    wq_p = _perm_even_odd(np.asarray(wq, np.float32), N_HEADS)
    wk_p = _perm_even_odd(np.asarray(wk, np.float32), N_KV)
    wv = np.asarray(wv, np.float32)
    wo_b = _bf16(wo)
    ident = np.eye(128, dtype=np.float32).astype(NPBF16)
    psw = _pswap().astype(NPBF16)
    md = _mdiag().astype(NPBF16)
    xT = _bf16(np.asarray(x, np.float32).reshape(B * s, D).T)

    in_maps = []
    for c in range(NCORES):
        wkv = np.concatenate([wk_p[:, c * HD:(c + 1) * HD],
                              wv[:, c * HD:(c + 1) * HD]], axis=1)
        im = {
            "xT": xT,
            "wq": _bf16(wq_p[:, c * QCOLS:(c + 1) * QCOLS]),
            "wkv": _bf16(wkv),
            "wo": wo_b,
            "c4": c4,
            "s4": s4,
            "ident": ident,
            "pswap": psw,
            "mdiag": md,
        }
        in_maps.append(im)
    return in_maps


def assemble_output(per_core_outs, s=S):
    rows_h = s // NCORES
    y = np.empty((B, s, D), dtype=np.float32)
    for c in range(NCORES):
        o = np.asarray(per_core_outs[c], np.float32)
        for b in range(B):
            y[b, c * rows_h:(c + 1) * rows_h, :] = \
                o[b * rows_h:(b + 1) * rows_h]
    return y


def _numpy_fallback(x, mask, wq, wk, wv, wo):
    """Reference-faithful numpy path for non-causal masks (never hit by
    the staged problem, kept for safety)."""
    x = np.asarray(x, np.float64)
    B_, S_, D_ = x.shape
    n_rep = N_HEADS // N_KV
    q = (x @ np.asarray(wq, np.float64)).reshape(B_, S_, N_HEADS, HD)
    k = (x @ np.asarray(wk, np.float64)).reshape(B_, S_, N_KV, HD)
    v = (x @ np.asarray(wv, np.float64)).reshape(B_, S_, N_KV, HD)
    freqs = 1.0 / (THETA ** (np.arange(0, HD, 2, dtype=np.float64) / HD))
    ang = np.arange(S_, dtype=np.float64)[:, None] * freqs[None, :]
    cos, sin = np.cos(ang), np.sin(ang)

    def rope(t):
        tr, ti = t[..., 0::2], t[..., 1::2]
        c = cos[None, :, None, :]
        sn = sin[None, :, None, :]
        o = np.empty_like(t)
        o[..., 0::2] = tr * c - ti * sn
        o[..., 1::2] = tr * sn + ti * c
        return o

    q, k = rope(q), rope(k)
    k = np.repeat(k, n_rep, axis=2)
    v = np.repeat(v, n_rep, axis=2)
    sc = np.einsum("bqhd,bkhd->bhqk", q, k) / math.sqrt(HD)
    sc = sc + np.asarray(mask, np.float64)
    sc -= sc.max(axis=-1, keepdims=True)
    p = np.exp(sc)
    p /= p.sum(axis=-1, keepdims=True)
    out = np.einsum("bhqk,bkhd->bqhd", p, v).reshape(B_, S_, N_HEADS * HD)
    return (out @ np.asarray(wo, np.float64)).astype(np.float32)


_GRAPH_CACHE = {}


def get_graph(s=S):
    if s not in _GRAPH_CACHE:
        _GRAPH_CACHE[s] = build_graph(s)
    return _GRAPH_CACHE[s]


def kernel(**inputs):
    x = np.asarray(inputs["x"], np.float32)
    mask = inputs["mask"]
    s = x.shape[1]
    if not _is_causal(mask, s):
        return _numpy_fallback(x, mask, inputs["wq"], inputs["wk"],
                               inputs["wv"], inputs["wo"])
    in_maps = make_in_maps(x, mask, inputs["wq"], inputs["wk"],
                           inputs["wv"], inputs["wo"], s=s)
    nc = get_graph(s)
    res = bass_utils.run_bass_kernel_spmd(nc, in_maps, core_ids=list(range(NCORES)))
    return assemble_output([res.results[c]["out"] for c in range(NCORES)], s=s)


# revision 41
# speedup vs baseline: 1.1273x; 1.1273x over previous
"""Distributed GQA attention (RoPE + causal mask + o_proj) on 8 TRN2 NeuronCores.

Sharding: 8-way tensor parallel over heads. Core c handles q heads
[4c, 4c+4) and kv head c (the matching GQA group) for BOTH batches.

v2 structure (vs the v1 baseline):
  - projections contract dc-outer so the PE starts ~3us in and each
    weight chunk's LDWEIGHTS is reused across 4-6 matmuls
  - score matmuls for the two heads of a pair are emitted interleaved so
    their (0,0)/(64,0) PE row-tiles execute concurrently (2x score rate)
  - per-head-pair exp runs as two [128,1024] ACTIVATEs per k-tile-pair;
    the next pair's score matmuls are emitted BEFORE this pair's AV
    matmuls so ScalarE (the attention pace-setter) never starves
  - softmax normalization reads the PSUM accumulator directly
    (reciprocal + gpsimd partition broadcast + one DVE multiply)
  - o_proj for batch 0 is emitted interleaved into batch 1's attention
    loops, filling the PE bubbles left by the exp dependency chain
  - ScalarE runs ONLY exp; all copies live on VectorE; DMA issue is
    spread across the sync/gpsimd/vector queues
"""

import os
import sys
import math

for _p in ("/opt/trn_rl_repo", "/root/.axon_site/_ro/trn_rl_repo"):
    if os.path.isdir(_p) and _p not in sys.path:
        sys.path.append(_p)

import numpy as np
import ml_dtypes

import concourse.bass as bass
import concourse.bacc as bacc
import concourse.tile as tile
import concourse.mybir as mybir
from concourse import bass_utils

FP32 = mybir.dt.float32
BF16 = mybir.dt.bfloat16
AF = mybir.ActivationFunctionType
ALU = mybir.AluOpType
PSUM = bass.MemorySpace.PSUM
NPBF16 = ml_dtypes.bfloat16

B = 2
D = 2048
S = 2048
HD = 64
N_HEADS = 32
N_KV = 8
NCORES = 8
HQ = N_HEADS // NCORES   # 4 local q heads
QCOLS = HQ * HD          # 256
NDC = D // 128           # 16 contraction chunks
NPAIR = HQ // 2          # 2 head pairs
THETA = 10000.0


def build_graph(s: int = S):
    """Build + compile the per-core SPMD graph (causal only)."""
    bs = B * s               # flattened rows
    nqb = s // 512           # q blocks per batch
    nkt = s // 128           # k tiles per batch
    rows_h = s // NCORES     # output rows per core per batch
    rt_m = min(128, rows_h)  # o_proj row-tile height
    nrt_h = rows_h // rt_m   # o_proj row tiles per batch half

    nc = bacc.Bacc("TRN2", target_bir_lowering=False, debug=False,
                   enable_asserts=True, num_devices=NCORES)

    xT_h = nc.dram_tensor("xT", [D, bs], BF16, kind="ExternalInput")
    wq_h = nc.dram_tensor("wq", [D, QCOLS], BF16, kind="ExternalInput")
    wkv_h = nc.dram_tensor("wkv", [D, 2 * HD], BF16, kind="ExternalInput")
    wo_h = nc.dram_tensor("wo", [D, D], BF16, kind="ExternalInput")
    c4_h = nc.dram_tensor("c4", [128, s], FP32, kind="ExternalInput")
    s4_h = nc.dram_tensor("s4", [128, s], FP32, kind="ExternalInput")
    id_h = nc.dram_tensor("ident", [128, 128], BF16, kind="ExternalInput")
    pw_h = nc.dram_tensor("pswap", [128, 128], BF16, kind="ExternalInput")
    md_h = nc.dram_tensor("mdiag", [128, 1024], BF16, kind="ExternalInput")
    out_h = nc.dram_tensor("out", [B * rows_h, D], FP32, kind="ExternalOutput")

    with tile.TileContext(nc) as tc:
        with tc.tile_pool(name="persist", bufs=1) as pp, \
             tc.tile_pool(name="dram", bufs=1, space="DRAM") as dramp:

            # ---- constants (tiles only; DMAs issued after the first x/w
            # chunk loads so they don't steal startup HBM bandwidth) ----
            c4 = pp.tile([128, s], FP32, tag="c4", name="c4t")
            s4 = pp.tile([128, s], FP32, tag="s4", name="s4t")
            ident = pp.tile([128, 128], BF16, tag="idb", name="identb")
            psw = pp.tile([128, 128], BF16, tag="pwb", name="pswapb")
            md = pp.tile([128, 1024], BF16, tag="mdb", name="mdb")

            def load_constants():
                # scalar DMA queue: idle during projections
                nc.scalar.dma_start(ident[:, :], id_h[:, :])
                nc.scalar.dma_start(psw[:, :], pw_h[:, :])
                nc.scalar.dma_start(c4[:, :], c4_h[:, :])
                nc.scalar.dma_start(s4[:, :], s4_h[:, :])
                nc.scalar.dma_start(md[:, :], md_h[:, :])

            # warm the exp table set early so the first attention exp
            # doesn't eat the ~2.7us ACT_TABLE_LOAD
            warm = pp.tile([1, 64], FP32, tag="warm", name="warm")
            nc.vector.memset(warm[0:1, :], 0.0)
            nc.scalar.activation(warm[0:1, :], warm[0:1, :], AF.Exp)
            # [1, 64] ones: stationary for the K=1 broadcast matmuls that
            # replicate the softmax reciprocals across partitions
            ones = pp.tile([1, HD], BF16, tag="ones", name="ones")
            nc.vector.memset(ones[0:1, :], 1.0)

            # ---- persistent activations ----
            qT = [pp.tile([128, bs], BF16, tag=f"qT{i}", name=f"qT{i}")
                  for i in range(NPAIR)]
            kTrep = [pp.tile([128, s], BF16, tag=f"kTr{i}", name=f"kTrep{i}")
                     for i in range(B)]
            vb = [[pp.tile([128, HD + 1], BF16, tag=f"vb{b}_{i}",
                           name=f"vb{b}_{i}") for i in range(nkt)]
                  for b in range(B)]
            # attn^T, one [64, bs] tile per local head (partition base 0)
            attnT = [pp.tile([64, bs], BF16, tag=f"aT{i}", name=f"attnT{i}")
                     for i in range(HQ)]

            bnc_in = [[dramp.tile([NCORES, 2 * HD, rows_h], BF16,
                                  tag=f"bin{b}_{hp}", name=f"bounce_in{b}_{hp}")
                       for hp in range(NPAIR)] for b in range(B)]
            bnc_out = [[dramp.tile([NCORES, 2 * HD, rows_h], BF16,
                                   tag=f"bout{b}_{hp}", name=f"bounce_out{b}_{hp}")
                        for hp in range(NPAIR)] for b in range(B)]

            # ================= phase P: projections + RoPE =================
            def rope(src_ps, swap_ps, scale, dst_ap, sl0, npart, rp, nm):
                """dst = (src * scale) .* c4 + (swap(src) * scale) .* s4."""
                cs = c4[0:npart, sl0:sl0 + 512]
                sn = s4[0:npart, sl0:sl0 + 512]
                m1 = rp.tile([npart, 512], FP32, tag=f"m1_{npart}", name=f"m1{nm}")
                nc.vector.scalar_tensor_tensor(m1[:, :], src_ps, scale, cs,
                                               ALU.mult, ALU.mult)
                m2 = rp.tile([npart, 512], FP32, tag=f"m2_{npart}", name=f"m2{nm}")
                nc.vector.scalar_tensor_tensor(m2[:, :], swap_ps, scale, sn,
                                               ALU.mult, ALU.mult)
                nc.vector.tensor_add(dst_ap, m1[:, :], m2[:, :])

            with tc.tile_pool(name="wres", bufs=1) as wrp, \
                 tc.tile_pool(name="xpool", bufs=18) as xp, \
                 tc.tile_pool(name="ropes", bufs=3) as rp, \
                 tc.tile_pool(name="qps_p", bufs=1, space=PSUM) as pq, \
                 tc.tile_pool(name="qsw_p", bufs=1, space=PSUM) as pqs, \
                 tc.tile_pool(name="kv_p", bufs=1, space=PSUM) as pkv, \
                 tc.tile_pool(name="tp_p", bufs=1, space=PSUM) as ptp:

                # per-dc weight chunk loads: small DMAs so the first matmul
                # can start ~3us in; spread across two queues
                wq_all = wrp.tile([128, NDC * QCOLS], BF16, tag="wqa",
                                  name="wq_all")
                wkv_all = wrp.tile([128, NDC * 2 * HD], BF16, tag="wkva",
                                   name="wkv_all")
                wqb = [wq_all[:, dc * QCOLS:(dc + 1) * QCOLS]
                       for dc in range(NDC)]
                wkvb = [wkv_all[:, dc * 2 * HD:(dc + 1) * 2 * HD]
                        for dc in range(NDC)]

                nxp = bs // 1024      # 1024-row compute slabs (2 rs slices)
                xtb2 = {}             # xj -> 16 [128, 2048] tiles (2 slabs)
                for xi in range(nxp):
                    x0 = xi * 1024
                    if xi % 2 == 0:
                        xj = xi // 2
                        tl = []
                        for dc in range(NDC):
                            if xi == 0:
                                # interleave weight chunks with x tiles so
                                # the dc-outer matmul stream starts at once
                                nc.sync.dma_start(
                                    wkvb[dc], wkv_h[dc * 128:(dc + 1) * 128, :])
                                nc.gpsimd.dma_start(
                                    wqb[dc], wq_h[dc * 128:(dc + 1) * 128, :])
                            xt = xp.tile([128, 2048], BF16, tag="xtb",
                                         name=f"xtb2_{xj}_{dc}")
                            eng = nc.sync if dc % 2 == 0 else nc.gpsimd
                            eng.dma_start(
                                xt[:, :],
                                xT_h[dc * 128:(dc + 1) * 128,
                                     xj * 2048:(xj + 1) * 2048])
                            tl.append(xt)
                        if xi == 0:
                            load_constants()
                        xtb2[xj] = tl
                    xh = (xi % 2) * 1024
                    xtb = [t[:, xh:xh + 1024] for t in xtb2[xi // 2]]

                    # dc-outer accumulation: all psum tiles live across the
                    # dc loop; each stationary is loaded once per dc
                    kvps = [pkv.tile([128, 512], FP32, tag=f"kvps{sub}",
                                     name=f"kvps{xi}_{sub}") for sub in range(2)]
                    qps = [[pq.tile([128, 512], FP32, tag=f"qps{hp}_{sub}",
                                    name=f"qps{xi}_{hp}_{sub}")
                            for sub in range(2)] for hp in range(NPAIR)]
                    for dc in range(NDC):
                        st = (dc == 0)
                        sp = (dc == NDC - 1)
                        for sub in range(2):
                            xsl = slice(sub * 512, sub * 512 + 512)
                            nc.tensor.matmul(kvps[sub][:, :], wkvb[dc],
                                             xtb[dc][:, xsl], start=st, stop=sp)
                        for hp in range(NPAIR):
                            for sub in range(2):
                                xsl = slice(sub * 512, sub * 512 + 512)
                                nc.tensor.matmul(
                                    qps[hp][sub][:, :],
                                    wqb[dc][:, hp * 128:(hp + 1) * 128],
                                    xtb[dc][:, xsl], start=st, stop=sp)

                    for sub in range(2):
                        rs = xi * 2 + sub
                        r0 = rs * 512          # flattened row offset
                        b = r0 // s            # batch of this slice
                        sl0 = r0 - b * s       # seq offset within batch

                        # k: rope + replicate into both 64-partition halves
                        ksb = rp.tile([64, 512], BF16, tag="ksb", name=f"ksb{rs}")
                        nc.vector.tensor_copy(ksb[:, :], kvps[sub][0:64, :])
                        ksw = pqs.tile([64, 512], FP32, tag="qsw", name=f"ksw{rs}")
                        nc.tensor.matmul(ksw[:, :], psw[0:64, 0:64], ksb[:, :],
                                         start=True, stop=True)
                        ktmp = rp.tile([64, 512], BF16, tag="ktmp", name=f"ktmp{rs}")
                        rope(kvps[sub][0:64, :], ksw[:, :], 1.0, ktmp[:, :],
                             sl0, 64, rp, f"k{rs}")
                        # scalar queue: keeps kTrep's completion semaphore
                        # free of late traffic, so the attention matmuls'
                        # (conservative, queue-ordinal) thresholds on it
                        # can never couple to collectives or staging
                        nc.scalar.dma_start(kTrep[b][0:64, sl0:sl0 + 512], ktmp[:, :])
                        nc.scalar.dma_start(kTrep[b][64:128, sl0:sl0 + 512], ktmp[:, :])
                        # v: PE-transpose vT to row-major + ones col
                        vts = rp.tile([128, 512], BF16, tag="vts", name=f"vts{rs}")
                        nc.vector.tensor_copy(vts[64:128, :], kvps[sub][64:128, :])
                        for rb in range(4):
                            kt = sl0 // 128 + rb
                            tps = ptp.tile([128, HD], BF16, tag="tps",
                                           name=f"vt{rs}_{rb}")
                            nc.tensor.transpose(tps[:, :],
                                                vts[64:128, rb * 128:(rb + 1) * 128],
                                                ident[64:128, 64:128])
                            nc.vector.tensor_copy(vb[b][kt][:, 0:HD], tps[:, :])
                            nc.vector.memset(vb[b][kt][:, HD:HD + 1], 1.0)

                        # q projection rope, one head pair at a time
                        for hp in range(NPAIR):
                            qsb = rp.tile([128, 512], BF16, tag="qsb",
                                          name=f"qsb{rs}_{hp}")
                            nc.vector.tensor_copy(qsb[:, :], qps[hp][sub][:, :])
                            qsw = pqs.tile([128, 512], FP32, tag="qsw",
                                           name=f"qsw{rs}_{hp}")
                            nc.tensor.matmul(qsw[:, :], psw[:, :], qsb[:, :],
                                             start=True, stop=True)
                            rope(qps[hp][sub][:, :], qsw[:, :], 0.125,
                                 qT[hp][:, r0:r0 + 512], sl0, 128, rp,
                                 f"q{rs}_{hp}")

            # wo tiles prefetch during attention (no deps besides SBUF slots)
            wop_ctx = tc.tile_pool(name="wob", bufs=1)
            wop = wop_ctx.__enter__()
            wot = []
            for ch in range(NDC):
                wt = wop.tile([128, D], BF16, tag=f"wob{ch}", name=f"wob{ch}")
                nc.sync.dma_start(wt[:, :], wo_h[ch * 128:(ch + 1) * 128, :])
                wot.append(wt)

            # ============ attention (+ per-batch-pair A2A) + o_proj ============
            with tc.tile_pool(name="sc_p", bufs=1, space=PSUM) as psc, \
                 tc.tile_pool(name="at_p", bufs=1, space=PSUM) as pat, \
                 tc.tile_pool(name="y_p", bufs=1, space=PSUM) as pyo, \
                 tc.tile_pool(name="rc_p", bufs=1, space=PSUM) as prc, \
                 tc.tile_pool(name="probs", bufs=3) as prp, \
                 tc.tile_pool(name="att2", bufs=1) as a2p, \
                 tc.tile_pool(name="yout", bufs=2) as yop, \
                 tc.tile_pool(name="norm", bufs=2) as nrm:

                def sc_mms(b, hp, qb, kp):
                    """4 score MMs for one k-pair. par-inner adjacency with
                    explicit tile_position so the (0,0)/(64,0) PE row-tile
                    pairs execute concurrently. Returns the two tiles."""
                    q0 = qb * 512
                    g0 = b * s + q0
                    out = [psc.tile([128, 1024], FP32, tag=f"sc{par}",
                                    name=f"sc{par}_{b}_{qb}_{hp}_{kp}")
                           for par in range(2)]
                    for i in range(2):
                        kt = 2 * kp + i
                        k0 = kt * 128
                        for par in range(2):
                            pr = par * 64
                            nc.tensor.matmul(
                                out[par][:, i * 512:(i + 1) * 512],
                                kTrep[b][pr:pr + 64, k0:k0 + 128],
                                qT[hp][pr:pr + 64, g0:g0 + 512],
                                start=True, stop=True,
                                tile_position=(pr, 0))
                    return out

                def attn_block(b, hp, qb, sc0, next_sc, filler=None):
                    """Emit one q-block of attention for head pair hp.

                    sc0: pre-emitted score tiles for this block's kp=0 (the
                    predecessor emitted them so ScalarE never bubbles at
                    block boundaries). next_sc() emits the successor
                    block's kp=0 right after this block's last exp.
                    filler() slots a few independent PE instructions
                    (o_proj) into the exp dependency gaps each iteration.
                    """
                    q0 = qb * 512            # seq offset within batch
                    g0 = b * s + q0          # flattened offset
                    kt_end = 4 * (qb + 1)
                    nkp = kt_end // 2
                    acc = [pat.tile([HD + 1, 512], FP32, tag=f"a{par}",
                                    name=f"a{par}_{b}_{qb}_{hp}")
                           for par in range(2)]
                    sc = sc0
                    for kp in range(nkp):
                        pbt = prp.tile([128, 2048], BF16, tag="pb",
                                       name=f"pb{b}_{qb}_{hp}_{kp}")
                        for par in range(2):
                            nc.scalar.activation(
                                pbt[:, par * 1024:(par + 1) * 1024],
                                sc[par][:, :], AF.Exp)
                        # prefetch the next k-pair's scores (or the next
                        # block's first) now that this pair's psum is free
                        if kp + 1 < nkp:
                            sc = sc_mms(b, hp, qb, kp + 1)
                        elif next_sc is not None:
                            next_sc()
                        for i in range(2):
                            kt = 2 * kp + i
                            if kt >= 4 * qb:      # diagonal band: 0/1 mask
                                delta = kt * 128 - q0
                                msl = md[:, 512 - delta:1024 - delta]
                                for par in range(2):
                                    c0 = par * 1024 + i * 512
                                    nc.vector.tensor_mul(
                                        pbt[:, c0:c0 + 512],
                                        pbt[:, c0:c0 + 512], msl)
                        for par in range(2):
                            for i in range(2):
                                kt = 2 * kp + i
                                c0 = par * 1024 + i * 512
                                nc.tensor.matmul(
                                    acc[par][:, :], vb[b][kt][:, :],
                                    pbt[:, c0:c0 + 512],
                                    start=(kt == 0), stop=(kt == kt_end - 1))
                        if filler is not None:
                            filler()

                    # normalization: DVE only copies PSUM out (releasing the
                    # acc banks fast) + tiny reciprocal; the broadcast and
                    # the big multiply run on GpSimd so the DVE queue never
                    # blocks on a cross-engine dependency
                    # normalization: DVE copies PSUM out (releasing the acc
                    # banks fast) + reciprocal; a K=1 PE matmul broadcasts
                    # the reciprocal across 64 partitions into PSUM (keeps
                    # gpsimd - busy with collectives - off this path)
                    asb, rdb = [], []
                    for par in range(2):
                        a = nrm.tile([HD, 512], FP32, tag=f"asb{par}",
                                     name=f"asb{b}_{qb}_{hp}_{par}")
                        nc.vector.tensor_copy(a[:, :], acc[par][0:HD, :])
                        rs = nrm.tile([1, 512], FP32, tag=f"rds{par}",
                                      name=f"rds{b}_{qb}_{hp}_{par}")
                        nc.vector.tensor_copy(rs[0:1, :], acc[par][HD:HD + 1, :])
                        r = nrm.tile([1, 512], BF16, tag=f"rd{par}",
                                     name=f"rd{b}_{qb}_{hp}_{par}")
                        rf = nrm.tile([1, 512], FP32, tag=f"rdf{par}",
                                      name=f"rdf{b}_{qb}_{hp}_{par}")
                        nc.vector.reciprocal_approx_fast(rf[0:1, :], rs[0:1, :])
                        nc.vector.tensor_copy(r[0:1, :], rf[0:1, :])
                        asb.append(a)
                        rdb.append(r)
                    rcb = prc.tile([128, 512], FP32, tag="rcb",
                                   name=f"rcb{b}_{qb}_{hp}")
                    for par in range(2):
                        nc.tensor.matmul(rcb[par * 64:(par + 1) * 64, :],
                                         ones[0:1, :], rdb[par][0:1, :],
                                         start=True, stop=True,
                                         tile_position=(0, par * 64))
                    for par in range(2):
                        head = hp * 2 + par
                        nc.vector.tensor_mul(attnT[head][:, g0:g0 + 512],
                                             asb[par][0:HD, :],
                                             rcb[par * 64:(par + 1) * 64, :])
                    # stage this q-block's two core-row slices for the A2A
                    for j in (2 * qb, 2 * qb + 1):
                        for par in range(2):
                            head = hp * 2 + par
                            nc.sync.dma_start(
                                bnc_in[b][hp][j, par * 64:(par + 1) * 64, :],
                                attnT[head][:, b * s + j * rows_h:
                                            b * s + (j + 1) * rows_h])

                def a2a(b, hp):
                    nc.gpsimd.collective_compute(
                        "AllToAll", ALU.bypass,
                        replica_groups=[list(range(NCORES))],
                        ins=[bnc_in[b][hp].opt()],
                        outs=[bnc_out[b][hp].opt()],
                    )

                att2 = {}

                def att2_load(b):
                    tiles = []
                    for i in range(NCORES):
                        for hp in range(NPAIR):
                            t = a2p.tile([128, rows_h], BF16,
                                         tag=f"at2_{b}_{i}_{hp}",
                                         name=f"att2_{b}_{i}_{hp}")
                            # gpsimd queue: the wait on the collective must
                            # not sit in front of DMAs other engines' deps
                            # are semaphore-coupled to (sync queue)
                            nc.gpsimd.dma_start(t[:, :], bnc_out[b][hp][i, :, :])
                            tiles.append(t)
                    att2[b] = tiles

                ngrp = (D // 512) * nrt_h      # o_proj groups per batch

                def oproj_steps(b):
                    """Generator: o_proj matmul micro-steps for batch b.
                    Each yield is one matmul; group boundaries allocate the
                    PSUM tile and emit the output store."""
                    for g in range(ngrp):
                        oc, rt = g // nrt_h, g % nrt_h
                        o0 = oc * 512
                        yps = pyo.tile([rt_m, 512], FP32, tag="y",
                                       name=f"y{b}_{oc}_{rt}")
                        for ch in range(NDC):
                            nc.tensor.matmul(
                                yps[:, :],
                                att2[b][ch][:, rt * rt_m:(rt + 1) * rt_m],
                                wot[ch][:, o0:o0 + 512],
                                start=(ch == 0), stop=(ch == NDC - 1))
                            yield
                        ysb = yop.tile([rt_m, 512], FP32, tag="ysb",
                                       name=f"ysb{b}_{oc}_{rt}")
                        nc.vector.tensor_copy(ysb[:, :], yps[:, :])
                        nc.sync.dma_start(
                            out_h[b * rows_h + rt * rt_m:
                                  b * rows_h + (rt + 1) * rt_m, o0:o0 + 512],
                            ysb[:, :])

                # batch-0 o_proj matmuls slot into batch-1 attention's PE
                # gaps (4 per k-pair step, after a delay so the A2A and
                # att2 loads have safely landed)
                osteps = oproj_steps(0)
                kp_slot = [0]

                def filler():
                    # disabled: interleaving o_proj into attention couples
                    # the PE queue to the collectives' completion and the
                    # induced cross-core skew costs more than the overlap
                    # saves; o_proj b0 instead runs contiguously after the
                    # last attention block, overlapping A2A(b1,hp1)
                    kp_slot[0] += 1

                blocks = [(b, hp, qb) for b in range(B)
                          for hp in range(NPAIR) for qb in range(nqb)]
                sc_pending = {0: sc_mms(*blocks[0], 0)}

                def make_next_sc(idx):
                    if idx + 1 >= len(blocks):
                        return None

                    def f():
                        nb, nhp, nq = blocks[idx + 1]
                        sc_pending[idx + 1] = sc_mms(nb, nhp, nq, 0)
                    return f

                for idx, (b, hp, qb) in enumerate(blocks):
                    attn_block(b, hp, qb, sc_pending.pop(idx),
                               make_next_sc(idx),
                               filler if b == 1 else None)
                    if qb == nqb - 1:
                        a2a(b, hp)
                        if b == 0 and hp == NPAIR - 1:
                            att2_load(0)
                for _ in osteps:               # drain any remainder
                    pass

                att2_load(1)
                for _ in oproj_steps(1):
                    pass

            wop_ctx.__exit__(None, None, None)

    nc.compile()
    return nc


# ===================== host side =====================

def _rope_tables(s):
    freqs = THETA ** (-np.arange(0, HD, 2, dtype=np.float64) / HD)   # [32]
    ang = np.arange(s, dtype=np.float64)[:, None] * freqs[None, :]   # [s, 32]
    cosT = np.cos(ang).T.astype(np.float32)                          # [32, s]
    sinT = np.sin(ang).T.astype(np.float32)
    c4 = np.tile(cosT, (4, 1))                                       # [128, s]
    s4 = np.tile(np.concatenate([-sinT, sinT], axis=0), (2, 1))      # [128, s]
    return np.ascontiguousarray(c4), np.ascontiguousarray(s4)


def _pswap():
    # permutation matrix: swap 32-halves within each 64 block (symmetric)
    p = np.zeros((128, 128), dtype=np.float32)
    for blk in range(2):
        for i in range(32):
            p[blk * 64 + i, blk * 64 + 32 + i] = 1.0
            p[blk * 64 + 32 + i, blk * 64 + i] = 1.0
    return p


def _mdiag():
    # keep[p, u] = 1 iff u >= p + 512 (sliced per diagonal tile offset)
    u = np.arange(1024)[None, :]
    p = np.arange(128)[:, None]
    return (u >= p + 512).astype(np.float32)


def _perm_even_odd(w, n_heads_w):
    # reorder each head's 64 columns: even indices first, then odd
    perm = np.concatenate([np.arange(0, HD, 2), np.arange(1, HD, 2)])
    wr = w.reshape(D, n_heads_w, HD)[:, :, perm]
    return np.ascontiguousarray(wr.reshape(D, n_heads_w * HD))


def _is_causal(mask, s):
    m = np.asarray(mask, dtype=np.float32).reshape(s, s)
    tri = np.tril(np.ones((s, s), dtype=bool))
    return bool(np.all(m[tri] == 0.0) and np.all(m[~tri] <= -1e8))


def _bf16(a):
    return np.ascontiguousarray(np.asarray(a, np.float32).astype(NPBF16))


def make_in_maps(x, mask, wq, wk, wv, wo, s=S):
    """Shard full inputs into 8 per-core input dicts."""
    c4, s4 = _rope_tables(s)
    wq_p = _perm_even_odd(np.asarray(wq, np.float32), N_HEADS)
    wk_p = _perm_even_odd(np.asarray(wk, np.float32), N_KV)
    wv = np.asarray(wv, np.float32)
    wo_b = _bf16(wo)
    ident = np.eye(128, dtype=np.float32).astype(NPBF16)
    psw = _pswap().astype(NPBF16)
    md = _mdiag().astype(NPBF16)
    xT = _bf16(np.asarray(x, np.float32).reshape(B * s, D).T)

    in_maps = []
    for c in range(NCORES):
        wkv = np.concatenate([wk_p[:, c * HD:(c + 1) * HD],
                              wv[:, c * HD:(c + 1) * HD]], axis=1)
        im = {
            "xT": xT,
            "wq": _bf16(wq_p[:, c * QCOLS:(c + 1) * QCOLS]),
            "wkv": _bf16(wkv),
            "wo": wo_b,
            "c4": c4,
            "s4": s4,
            "ident": ident,
            "pswap": psw,
            "mdiag": md,
        }
        in_maps.append(im)
    return in_maps


def assemble_output(per_core_outs, s=S):
    rows_h = s // NCORES
    y = np.empty((B, s, D), dtype=np.float32)
    for c in range(NCORES):
        o = np.asarray(per_core_outs[c], np.float32)
        for b in range(B):
            y[b, c * rows_h:(c + 1) * rows_h, :] = \
                o[b * rows_h:(b + 1) * rows_h]
    return y


def _numpy_fallback(x, mask, wq, wk, wv, wo):
    """Reference-faithful numpy path for non-causal masks (never hit by
    the staged problem, kept for safety)."""
    x = np.asarray(x, np.float64)
    B_, S_, D_ = x.shape
    n_rep = N_HEADS // N_KV
    q = (x @ np.asarray(wq, np.float64)).reshape(B_, S_, N_HEADS, HD)
    k = (x @ np.asarray(wk, np.float64)).reshape(B_, S_, N_KV, HD)
    v = (x @ np.asarray(wv, np.float64)).reshape(B_, S_, N_KV, HD)
    freqs = 1.0 / (THETA ** (np.arange(0, HD, 2, dtype=np.float64) / HD))
    ang = np.arange(S_, dtype=np.float64)[:, None] * freqs[None, :]
    cos, sin = np.cos(ang), np.sin(ang)

    def rope(t):
        tr, ti = t[..., 0::2], t[..., 1::2]
        c = cos[None, :, None, :]
        sn = sin[None, :, None, :]
        o = np.empty_like(t)
        o[..., 0::2] = tr * c - ti * sn
        o[..., 1::2] = tr * sn + ti * c
        return o

    q, k = rope(q), rope(k)
    k = np.repeat(k, n_rep, axis=2)
    v = np.repeat(v, n_rep, axis=2)
    sc = np.einsum("bqhd,bkhd->bhqk", q, k) / math.sqrt(HD)
    sc = sc + np.asarray(mask, np.float64)
    sc -= sc.max(axis=-1, keepdims=True)
    p = np.exp(sc)
    p /= p.sum(axis=-1, keepdims=True)
    out = np.einsum("bhqk,bkhd->bqhd", p, v).reshape(B_, S_, N_HEADS * HD)
    return (out @ np.asarray(wo, np.float64)).astype(np.float32)


_GRAPH_CACHE = {}


def get_graph(s=S):
    if s not in _GRAPH_CACHE:
        _GRAPH_CACHE[s] = build_graph(s)
    return _GRAPH_CACHE[s]


def kernel(**inputs):
    x = np.asarray(inputs["x"], np.float32)
    mask = inputs["mask"]
    s = x.shape[1]
    if not _is_causal(mask, s):
        return _numpy_fallback(x, mask, inputs["wq"], inputs["wk"],
                               inputs["wv"], inputs["wo"])
    in_maps = make_in_maps(x, mask, inputs["wq"], inputs["wk"],
                           inputs["wv"], inputs["wo"], s=s)
    nc = get_graph(s)
    res = bass_utils.run_bass_kernel_spmd(nc, in_maps, core_ids=list(range(NCORES)))
    return assemble_output([res.results[c]["out"] for c in range(NCORES)], s=s)
